# revision 8
# baseline (speedup 1.0000x reference)
"""Trainium2 Bass kernel for the ChunkedSIEVE model (segment_reduce).

Math (see reference):
  x[b,v,:]  = tanh(feat[b,v,:] @ W_feat + b_feat + pos[b,v]*1e-6 * w_pos)
              + gene_table[gene_ids[b,v]]
  emb[b]    = mean_v x[b,v,:]                      (mask is all ones)
  scores[b] = tanh(emb @ W_att1 + b_att1) @ W_att2 (+ b_att2, cancels in softmax)
  per-sample (8 contiguous chunks) softmax over scores -> w
  out[s]    = sum_b w[b] * (emb[b] @ W_cls) + b_cls

Strategy: data-parallel over chunks, 256 chunks (32 samples) per core.
Everything stays in [D x row] layout on-chip:
  - PE computes z = [W_feat; w_pos]^T @ [feat; pos] in bf16 (K=65).
  - ACT applies tanh straight out of PSUM in [128, 1536] tiles.
  - DVE folds the per-chunk V-sum via tensor_scalar accum_out (4x bf16).
  - The gene-table term is a dense matmul: per-chunk gene histograms
    (fp8, exact small ints) against the bf16 gene table, accumulated in
    PSUM over 157 K-tiles of 128 genes, interleaved with the feature
    supertiles so PE fills the slack while ACT crunches tanh.
  - A tiny pair of matmuls projects t1 by [W_att1 | W_cls]/V, then the
    per-sample softmax runs with samples on partitions ([32, 8] layout).
"""

import functools
import os
import sys

import numpy as np

for _p in ("/opt/trn_rl_repo",):
    if _p not in sys.path and os.path.isdir(_p):
        sys.path.insert(0, _p)

import ml_dtypes  # noqa: E402

import concourse.bass as bass  # noqa: E402
import concourse.tile as tile  # noqa: E402
from concourse import bacc, mybir  # noqa: E402
from concourse.bass_utils import run_bass_kernel_spmd  # noqa: E402
from contextlib import ExitStack  # noqa: E402

F32 = mybir.dt.float32
BF16 = mybir.dt.bfloat16
FP8 = mybir.dt.float8e4
AF = mybir.ActivationFunctionType
ALU = mybir.AluOpType
AX = mybir.AxisListType

B, V, F, D, G, S = 2048, 256, 64, 256, 20000, 256
POS_SCALE = 1e-6
NCORES = 8
BC = B // NCORES          # 256 chunks per core
RC = BC * V               # 65536 rows per core
SC = S // NCORES          # 32 samples per core
K8 = B // S               # 8 chunks per sample
KIN = F + 1               # 65 = features + position row
KT = 157                  # gene K-tiles of 128
G_PAD = KT * 128          # 20096
ROWS_ST = 1536            # rows per supertile (6 chunks)
NST = (RC + ROWS_ST - 1) // ROWS_ST   # 43 (42 full + 1 of 1024 rows)
FEED = ROWS_ST * 4        # featT DMA chunk: 4 supertiles
NFEED = (RC + FEED - 1) // FEED       # 11
GENE_PER_SLOT = 4         # gene K-tiles emitted per supertile slot


def _emit(nc, tc, featT, countP, geneP, w65, bfeat, psc, batt1, watt2, bcls,
          out):
    ctx = ExitStack()
    with ctx:
        const = ctx.enter_context(tc.tile_pool(name="const", bufs=1))
        big = ctx.enter_context(tc.tile_pool(name="big", bufs=1))
        feat_p = ctx.enter_context(tc.tile_pool(name="feat", bufs=3))
        xt_p = ctx.enter_context(tc.tile_pool(name="xt", bufs=3))
        fold_p = ctx.enter_context(tc.tile_pool(name="fold", bufs=2))
        acc = ctx.enter_context(tc.tile_pool(name="acc", bufs=1))
        fpsum = ctx.enter_context(tc.tile_pool(name="fpsum", bufs=2,
                                               space="PSUM"))
        gpsum = ctx.enter_context(tc.tile_pool(name="gpsum", bufs=1,
                                               space="PSUM"))
        small = ctx.enter_context(tc.tile_pool(name="small", bufs=1))
        dram_p = ctx.enter_context(tc.tile_pool(name="dram", bufs=1,
                                                space="DRAM"))

        # ---- constants ----
        w65_t = const.tile([KIN, D], BF16)
        nc.sync.dma_start(w65_t[:, :], w65[:, :])
        bf_t = const.tile([128, 2], F32)
        nc.sync.dma_start(bf_t[:, 0:1], bfeat[0:128, :])
        nc.sync.dma_start(bf_t[:, 1:2], bfeat[128:256, :])
        psc_t0 = const.tile([128, KIN], F32)
        psc_t1 = const.tile([128, KIN], F32)
        nc.sync.dma_start(psc_t0[:, :], psc[0:128, :])
        nc.sync.dma_start(psc_t1[:, :], psc[128:256, :])
        batt1_t = const.tile([64, 1], F32)
        nc.sync.dma_start(batt1_t[:, :], batt1[:, :])
        watt2_t = const.tile([64, 1], F32)
        nc.sync.dma_start(watt2_t[:, :], watt2[:, :])
        bcls_t = const.tile([1, 1], F32)
        nc.sync.dma_start(bcls_t[:, :], bcls[:, :])

        # ---- big resident slabs: gene table (bf16) + histograms (fp8) ----
        geneS = big.tile([128, KT * D], BF16)
        countS = big.tile([128, KT * BC], FP8)
        NSLAB = 4
        kb = [0, 40, 80, 120, KT]
        for sl in range(NSLAB):
            k0, k1 = kb[sl], kb[sl + 1]
            nc.sync.dma_start(geneS[:, k0 * D:k1 * D], geneP[:, k0 * D:k1 * D])
            nc.sync.dma_start(countS[:, k0 * BC:k1 * BC],
                              countP[:, k0 * BC:k1 * BC])

        # per-(D-half) accumulators
        gacc = [gpsum.tile([128, BC], F32, tag=f"gps_{h}", name=f"gacc{h}")
                for h in range(2)]
        t1 = [acc.tile([128, BC], F32, tag=f"t1_{h}", name=f"t1_{h}")
              for h in range(2)]

        # ---- main loop over supertiles ----
        ften = [None] * NFEED

        def fetch_feed(ci):
            if ci < NFEED and ften[ci] is None:
                cols = min(FEED, RC - ci * FEED)
                ft = feat_p.tile([KIN, FEED], BF16, tag="ft", name="ft")
                nc.sync.dma_start(ft[:, :cols],
                                  featT[:, ci * FEED:ci * FEED + cols])
                ften[ci] = ft

        fetch_feed(0)
        fetch_feed(1)
        gene_k = 0
        for t in range(NST):
            r0 = t * ROWS_ST
            rows = min(ROWS_ST, RC - r0)
            ci = r0 // FEED
            fetch_feed(ci + 2)
            ft = ften[ci]
            off = r0 - ci * FEED
            # gene matmuls first: no ACT dependency, so they fill the PE
            # FIFO head while the feature psum buffer waits on ACT (keeps
            # the PE busy-burst contiguous -> HAM stays at K=8/8)
            ek = min(KT, gene_k + GENE_PER_SLOT)
            for k in range(gene_k, ek):
                for h in range(2):
                    nc.tensor.matmul(
                        gacc[h][:, :],
                        geneS[:, k * D + h * 128:k * D + h * 128 + 128],
                        countS[:, k * BC:(k + 1) * BC],
                        start=(k == 0),
                        stop=(k == KT - 1),
                    )
            gene_k = ek
            for h in range(2):
                ps = fpsum.tile([128, ROWS_ST], F32, tag="ps", name="ps")
                for q in range(rows // 512):
                    nc.tensor.matmul(
                        ps[:, q * 512:(q + 1) * 512],
                        w65_t[:, h * 128:(h + 1) * 128],
                        ft[:, off + q * 512:off + (q + 1) * 512],
                        start=True,
                        stop=True,
                    )
                xt = xt_p.tile([128, ROWS_ST], BF16, tag="xt", name="xt")
                nc.scalar.activation(xt[:, :rows], ps[:, :rows], AF.Tanh,
                                     bias=bf_t[:, h:h + 1])
                # per-chunk V-sum: two bf16 tree folds at DVE 2x mode, then
                # a strided 1x reduce over the remaining 64 per chunk
                nch = rows // V
                xv = xt[:, :rows].rearrange("p (c v) -> p c v", v=V)
                y1 = fold_p.tile([128, 6 * 128], BF16, tag="y1", name="y1")
                y1v = y1[:, :nch * 128].rearrange("p (c v) -> p c v", v=128)
                nc.vector.tensor_add(y1v, xv[:, :, 0:128], xv[:, :, 128:256])
                y2 = fold_p.tile([128, 6 * 64], BF16, tag="y2", name="y2")
                y2v = y2[:, :nch * 64].rearrange("p (c v) -> p c v", v=64)
                # second fold on the otherwise-idle GPSIMD to unload DVE
                nc.gpsimd.tensor_add(y2v, y1v[:, :, 0:64], y1v[:, :, 64:128])
                nc.vector.reduce_sum(t1[h][:, r0 // V:r0 // V + nch], y2v,
                                     axis=AX.X)

        # ---- combine + project:  h[c, b] = sum_d P[d, c] * (t1+g)[d, b] ----
        for h in range(2):
            nc.vector.tensor_add(t1[h][:, :], t1[h][:, :], gacc[h][:, :])

        psH = gpsum.tile([128, BC], F32, tag="gps_0", name="psH")
        hv = psH[0:KIN, :]
        nc.tensor.matmul(hv, psc_t0[:, :], t1[0][:, :], start=True, stop=False)
        nc.tensor.matmul(hv, psc_t1[:, :], t1[1][:, :], start=False, stop=True)

        u_t = small.tile([64, BC], F32)
        nc.scalar.activation(u_t[:, :], psH[0:64, :], AF.Tanh,
                             bias=batt1_t[:, :])
        a_t = small.tile([1, BC], F32)
        # a = emb @ W_cls / V + b_cls  (adding b_cls here is fine: sum w = 1)
        nc.scalar.activation(a_t[:, :], psH[64:65, :], AF.Identity,
                             bias=bcls_t[:, :])

        psS = gpsum.tile([128, BC], F32, tag="gps_1", name="psS")
        nc.tensor.matmul(psS[0:1, :], watt2_t[:, :], u_t[:, :],
                         start=True, stop=True)
        s_t = small.tile([1, BC], F32)
        nc.vector.tensor_copy(s_t[:, :], psS[0:1, :])

        # ---- reshape [1, BC] -> [SC, K8] via DRAM round trip ----
        scr_s = dram_p.tile([1, BC], F32)
        scr_a = dram_p.tile([1, BC], F32)
        nc.sync.dma_start(scr_s[:, :], s_t[:, :])
        nc.sync.dma_start(scr_a[:, :], a_t[:, :])
        s32 = small.tile([SC, K8], F32)
        a32 = small.tile([SC, K8], F32)
        nc.sync.dma_start(
            s32[:, :], scr_s[:, :].rearrange("o (s k) -> (o s) k", k=K8))
        nc.sync.dma_start(
            a32[:, :], scr_a[:, :].rearrange("o (s k) -> (o s) k", k=K8))

        # ---- per-sample softmax over the 8 chunks, samples on partitions ----
        smax = small.tile([SC, 1], F32)
        nc.vector.reduce_max(smax[:, :], s32[:, :], axis=AX.X)
        es = small.tile([SC, K8], F32)
        nc.vector.tensor_scalar(es[:, :], s32[:, :], smax[:, :], None,
                                op0=ALU.subtract)
        e_t = small.tile([SC, K8], F32)
        nc.scalar.activation(e_t[:, :], es[:, :], AF.Exp)
        ssum = small.tile([SC, 1], F32)
        nc.vector.reduce_sum(ssum[:, :], e_t[:, :], axis=AX.X)
        rec = small.tile([SC, 1], F32)
        nc.vector.reciprocal(rec[:, :], ssum[:, :])
        wa = small.tile([SC, K8], F32)
        nc.vector.tensor_mul(wa[:, :], e_t[:, :], a32[:, :])
        was = small.tile([SC, 1], F32)
        nc.vector.reduce_sum(was[:, :], wa[:, :], axis=AX.X)
        o_t = small.tile([SC, 1], F32)
        nc.vector.tensor_mul(o_t[:, :], was[:, :], rec[:, :])
        nc.sync.dma_start(out[:, :], o_t[:, :])


@functools.lru_cache(maxsize=1)
def _build():
    nc = bacc.Bacc(
        "TRN2",
        target_bir_lowering=False,
        debug=False,
        enable_asserts=False,
        num_devices=NCORES,
    )
    featT = nc.dram_tensor("featT", [KIN, RC], BF16, kind="ExternalInput")
    countP = nc.dram_tensor("countP", [128, KT * BC], FP8,
                            kind="ExternalInput")
    geneP = nc.dram_tensor("geneP", [128, KT * D], BF16, kind="ExternalInput")
    w65 = nc.dram_tensor("w65", [KIN, D], BF16, kind="ExternalInput")
    bfeat = nc.dram_tensor("bfeat", [D, 1], F32, kind="ExternalInput")
    psc = nc.dram_tensor("psc", [D, KIN], F32, kind="ExternalInput")
    batt1 = nc.dram_tensor("batt1", [64, 1], F32, kind="ExternalInput")
    watt2 = nc.dram_tensor("watt2", [64, 1], F32, kind="ExternalInput")
    bcls = nc.dram_tensor("bcls", [1, 1], F32, kind="ExternalInput")
    out = nc.dram_tensor("out", [SC, 1], F32, kind="ExternalOutput")
    with tile.TileContext(nc) as tc:
        _emit(nc, tc, featT.ap(), countP.ap(), geneP.ap(), w65.ap(),
              bfeat.ap(), psc.ap(), batt1.ap(), watt2.ap(), bcls.ap(),
              out.ap())
    nc.compile()
    return nc


def _prep_inputs(features, positions, gene_ids, mask, original_sample_indices,
                 W_feat, b_feat, gene_table, w_pos,
                 W_att1, b_att1, W_att2, b_att2, W_cls, b_cls):
    features = np.asarray(features, np.float32)
    positions = np.asarray(positions)
    gene_ids = np.asarray(gene_ids)
    fp8np = mybir.dt.np(FP8)

    featT_full = np.empty((KIN, B * V), np.float32)
    featT_full[:F] = features.reshape(B * V, F).T
    featT_full[F] = positions.reshape(-1).astype(np.float32) * POS_SCALE
    featT_bf = featT_full.astype(ml_dtypes.bfloat16)

    gp = np.zeros((G_PAD, D), np.float32)
    gp[:G] = np.asarray(gene_table, np.float32)
    gene_packed = np.ascontiguousarray(
        gp.reshape(KT, 128, D).transpose(1, 0, 2).reshape(128, KT * D)
        .astype(ml_dtypes.bfloat16))

    w65v = np.concatenate(
        [np.asarray(W_feat, np.float32),
         np.asarray(w_pos, np.float32)[None, :]], axis=0
    ).astype(ml_dtypes.bfloat16)
    pscv = np.ascontiguousarray(
        np.concatenate([np.asarray(W_att1, np.float32),
                        np.asarray(W_cls, np.float32)], axis=1) / V)
    bfeatv = np.ascontiguousarray(np.asarray(b_feat, np.float32)[:, None])
    batt1v = np.ascontiguousarray(np.asarray(b_att1, np.float32)[:, None])
    watt2v = np.ascontiguousarray(np.asarray(W_att2, np.float32))
    bclsv = np.asarray(b_cls, np.float32).reshape(1, 1)

    ids = gene_ids.reshape(B, V).astype(np.int64)
    chunk_base = np.arange(BC, dtype=np.int64)[:, None] * G_PAD
    in_maps = []
    for c in range(NCORES):
        ids_c = ids[c * BC:(c + 1) * BC]
        flat = (chunk_base + ids_c).ravel()
        counts = np.bincount(flat, minlength=BC * G_PAD).reshape(BC, G_PAD)
        if counts.max() > 16:
            return None  # fp8 would be inexact; caller falls back
        counts_packed = np.ascontiguousarray(
            counts.T.reshape(KT, 128, BC).transpose(1, 0, 2)
            .reshape(128, KT * BC).astype(fp8np))
        in_maps.append({
            "featT": np.ascontiguousarray(featT_bf[:, c * RC:(c + 1) * RC]),
            "countP": counts_packed,
            "geneP": gene_packed,
            "w65": w65v,
            "bfeat": bfeatv,
            "psc": pscv,
            "batt1": batt1v,
            "watt2": watt2v,
            "bcls": bclsv,
        })
    return in_maps


def _run(inputs, trace=False, **kw):
    nc = _build()
    in_maps = _prep_inputs(**inputs)
    if in_maps is None:
        return None, None
    res = run_bass_kernel_spmd(
        nc, in_maps, core_ids=list(range(NCORES)), trace=trace, **kw)
    outv = np.concatenate(
        [np.asarray(res.results[c]["out"], np.float32) for c in range(NCORES)],
        axis=0)
    return outv, res


def _numpy_fallback(features, positions, gene_ids, mask,
                    original_sample_indices, W_feat, b_feat, gene_table,
                    w_pos, W_att1, b_att1, W_att2, b_att2, W_cls, b_cls):
    features = np.asarray(features, np.float32)
    mask_f = np.asarray(mask, np.float32)
    pos = np.asarray(positions).astype(np.float32) * POS_SCALE
    x = np.tanh(features @ np.asarray(W_feat, np.float32)
                + np.asarray(b_feat, np.float32)
                + pos[..., None] * np.asarray(w_pos, np.float32))
    x = x + np.asarray(gene_table, np.float32)[np.asarray(gene_ids)]
    denom = np.maximum(mask_f.sum(-1, keepdims=True), 1.0)
    emb = (x * mask_f[..., None]).sum(axis=1) / denom
    scores = (np.tanh(emb @ np.asarray(W_att1, np.float32)
                      + np.asarray(b_att1, np.float32))
              @ np.asarray(W_att2, np.float32)
              + np.asarray(b_att2, np.float32))[:, 0]
    seg = np.asarray(original_sample_indices).astype(np.int64)
    smax = np.full(S, -np.inf, np.float32)
    np.maximum.at(smax, seg, scores)
    e = np.exp(scores - smax[seg])
    ssum = np.zeros(S, np.float32)
    np.add.at(ssum, seg, e)
    w = e / ssum[seg]
    agg = np.zeros((S, D), np.float32)
    np.add.at(agg, seg, emb * w[:, None])
    return agg @ np.asarray(W_cls, np.float32) + np.asarray(b_cls, np.float32)


def kernel(**inputs):
    mask = np.asarray(inputs["mask"])
    seg = np.asarray(inputs["original_sample_indices"]).astype(np.int64)
    expected_seg = np.arange(B) // K8
    if not mask.all() or not np.array_equal(seg, expected_seg):
        return _numpy_fallback(**inputs)
    outv, _ = _run(inputs)
    if outv is None:
        return _numpy_fallback(**inputs)
    return outv


# revision 14
# speedup vs baseline: 1.1982x; 1.1982x over previous
"""Trainium2 Bass kernel for the ChunkedSIEVE model (segment_reduce).

Math (see reference):
  x[b,v,:]  = tanh(feat[b,v,:] @ W_feat + b_feat + pos[b,v]*1e-6 * w_pos)
              + gene_table[gene_ids[b,v]]
  emb[b]    = mean_v x[b,v,:]                      (mask is all ones)
  scores[b] = tanh(emb @ W_att1 + b_att1) @ W_att2 (+ b_att2, cancels in softmax)
  per-sample (8 contiguous chunks) softmax over scores -> w
  out[s]    = sum_b w[b] * (emb[b] @ W_cls) + b_cls

Strategy: data-parallel over chunks, 256 chunks (32 samples) per core.
Everything stays in [D x row] layout on-chip:
  - PE computes z = [W_feat; w_pos]^T @ [feat; pos] in bf16 (K=65).
  - ACT applies tanh straight out of PSUM in [128, 1536] tiles.
  - DVE folds the per-chunk V-sum via tensor_scalar accum_out (4x bf16).
  - The gene-table term is a dense matmul: per-chunk gene histograms
    (fp8, exact small ints) against the bf16 gene table, accumulated in
    PSUM over 157 K-tiles of 128 genes, interleaved with the feature
    supertiles so PE fills the slack while ACT crunches tanh.
  - A tiny pair of matmuls projects t1 by [W_att1 | W_cls]/V, then the
    per-sample softmax runs with samples on partitions ([32, 8] layout).
"""

import functools
import os
import sys

import numpy as np

for _p in ("/opt/trn_rl_repo",):
    if _p not in sys.path and os.path.isdir(_p):
        sys.path.insert(0, _p)

import ml_dtypes  # noqa: E402

import concourse.bass as bass  # noqa: E402
import concourse.tile as tile  # noqa: E402
from concourse import bacc, mybir  # noqa: E402
from concourse.bass_utils import run_bass_kernel_spmd  # noqa: E402
from contextlib import ExitStack  # noqa: E402

F32 = mybir.dt.float32
BF16 = mybir.dt.bfloat16
FP8 = mybir.dt.float8e4
AF = mybir.ActivationFunctionType
ALU = mybir.AluOpType
AX = mybir.AxisListType

B, V, F, D, G, S = 2048, 256, 64, 256, 20000, 256
POS_SCALE = 1e-6
NCORES = 8
BC = B // NCORES          # 256 chunks per core
RC = BC * V               # 65536 rows per core
SC = S // NCORES          # 32 samples per core
K8 = B // S               # 8 chunks per sample
KIN = F + 1               # 65 = features + position row
KT = 157                  # gene K-tiles of 128
G_PAD = KT * 128          # 20096
ROWS_ST = 1536            # rows per supertile (6 chunks)
NST = (RC + ROWS_ST - 1) // ROWS_ST   # 43 (42 full + 1 of 1024 rows)
FEED = ROWS_ST * 4        # featT DMA chunk: 4 supertiles
NFEED = (RC + FEED - 1) // FEED       # 11
GENE_PER_SLOT = 4         # gene K-tiles emitted per supertile slot


def _emit(nc, tc, featT, countP, geneP, w65, bfeat, psc, batt1, watt2, bcls,
          out):
    ctx = ExitStack()
    with ctx:
        const = ctx.enter_context(tc.tile_pool(name="const", bufs=1))
        big = ctx.enter_context(tc.tile_pool(name="big", bufs=1))
        feat_p = ctx.enter_context(tc.tile_pool(name="feat", bufs=3))
        xt_p = ctx.enter_context(tc.tile_pool(name="xt", bufs=3))
        fold_p = ctx.enter_context(tc.tile_pool(name="fold", bufs=2))
        acc = ctx.enter_context(tc.tile_pool(name="acc", bufs=1))
        fpsum = ctx.enter_context(tc.tile_pool(name="fpsum", bufs=2,
                                               space="PSUM"))
        gpsum = ctx.enter_context(tc.tile_pool(name="gpsum", bufs=1,
                                               space="PSUM"))
        small = ctx.enter_context(tc.tile_pool(name="small", bufs=1))
        dram_p = ctx.enter_context(tc.tile_pool(name="dram", bufs=1,
                                                space="DRAM"))

        # ---- constants ----
        w65_t = const.tile([KIN, D], BF16)
        nc.sync.dma_start(w65_t[:, :], w65[:, :])
        bf_t = const.tile([128, 2], F32)
        nc.sync.dma_start(bf_t[:, 0:1], bfeat[0:128, :])
        nc.sync.dma_start(bf_t[:, 1:2], bfeat[128:256, :])
        psc_t0 = const.tile([128, KIN], F32)
        psc_t1 = const.tile([128, KIN], F32)
        nc.sync.dma_start(psc_t0[:, :], psc[0:128, :])
        nc.sync.dma_start(psc_t1[:, :], psc[128:256, :])
        batt1_t = const.tile([64, 1], F32)
        nc.sync.dma_start(batt1_t[:, :], batt1[:, :])
        watt2_t = const.tile([64, 1], F32)
        nc.sync.dma_start(watt2_t[:, :], watt2[:, :])
        bcls_t = const.tile([1, 1], F32)
        nc.sync.dma_start(bcls_t[:, :], bcls[:, :])

        # ---- big resident slabs: gene table (bf16) + histograms (fp8) ----
        geneS = big.tile([128, KT * D], BF16)
        countS = big.tile([128, KT * BC], FP8)
        NSLAB = 4
        kb = [0, 40, 80, 120, KT]
        for sl in range(NSLAB):
            k0, k1 = kb[sl], kb[sl + 1]
            nc.sync.dma_start(geneS[:, k0 * D:k1 * D], geneP[:, k0 * D:k1 * D])
            nc.sync.dma_start(countS[:, k0 * BC:k1 * BC],
                              countP[:, k0 * BC:k1 * BC])

        # per-(D-half) accumulators; both gene halves share one PSUM bank
        gboth = gpsum.tile([128, 2 * BC], F32, tag="gps", name="gboth")
        gacc = [gboth[:, h * BC:(h + 1) * BC] for h in range(2)]
        t1 = [acc.tile([128, BC], F32, tag=f"t1_{h}", name=f"t1_{h}")
              for h in range(2)]

        # dummy-matmul target: keeps the PE array active through the short
        # ACT-gated waits so HAM never re-throttles the PE clock to 4/8
        dum = gpsum.tile([128, 64], F32, tag="dummy", name="dum")

        def pe_filler(n):
            for _ in range(n):
                nc.tensor.matmul(dum[0:32, :], w65_t[:, 0:32], w65_t[:, 0:64],
                                 start=True, stop=True)

        pe_filler(24)  # warm up HAM while the first feature chunks stream in

        # ---- main loop over supertiles ----
        ften = [None] * NFEED

        def fetch_feed(ci):
            if ci < NFEED and ften[ci] is None:
                cols = min(FEED, RC - ci * FEED)
                ft = feat_p.tile([KIN, FEED], BF16, tag="ft", name="ft")
                nc.sync.dma_start(ft[:, :cols],
                                  featT[:, ci * FEED:ci * FEED + cols])
                ften[ci] = ft

        fetch_feed(0)
        fetch_feed(1)
        gene_k = 0
        for t in range(NST):
            r0 = t * ROWS_ST
            rows = min(ROWS_ST, RC - r0)
            ci = r0 // FEED
            fetch_feed(ci + 2)
            ft = ften[ci]
            off = r0 - ci * FEED
            # gene matmuls first: no ACT dependency, so they fill the PE
            # FIFO head while the feature psum buffer waits on ACT (keeps
            # the PE busy-burst contiguous -> HAM stays at K=8/8)
            pe_filler(8)
            ek = min(KT, gene_k + GENE_PER_SLOT)
            for k in range(gene_k, ek):
                for h in range(2):
                    nc.tensor.matmul(
                        gacc[h],
                        geneS[:, k * D + h * 128:k * D + h * 128 + 128],
                        countS[:, k * BC:(k + 1) * BC],
                        start=(k == 0),
                        stop=(k == KT - 1),
                        skip_group_check=True,
                    )
            gene_k = ek
            for h in range(2):
                ps = fpsum.tile([128, ROWS_ST], F32, tag="ps", name="ps")
                for q in range(rows // 512):
                    nc.tensor.matmul(
                        ps[:, q * 512:(q + 1) * 512],
                        w65_t[:, h * 128:(h + 1) * 128],
                        ft[:, off + q * 512:off + (q + 1) * 512],
                        start=True,
                        stop=True,
                    )
                xt = xt_p.tile([128, ROWS_ST], BF16, tag="xt", name="xt")
                nc.scalar.activation(xt[:, :rows], ps[:, :rows], AF.Tanh,
                                     bias=bf_t[:, h:h + 1])
                # per-chunk V-sum: two bf16 tree folds at DVE 2x mode, then
                # a strided 1x reduce over the remaining 64 per chunk
                nch = rows // V
                xv = xt[:, :rows].rearrange("p (c v) -> p c v", v=V)
                y1 = fold_p.tile([128, 6 * 128], BF16, tag="y1", name="y1")
                y1v = y1[:, :nch * 128].rearrange("p (c v) -> p c v", v=128)
                nc.vector.tensor_add(y1v, xv[:, :, 0:128], xv[:, :, 128:256])
                y2 = fold_p.tile([128, 6 * 64], BF16, tag="y2", name="y2")
                y2v = y2[:, :nch * 64].rearrange("p (c v) -> p c v", v=64)
                nc.vector.tensor_add(y2v, y1v[:, :, 0:64], y1v[:, :, 64:128])
                nc.vector.reduce_sum(t1[h][:, r0 // V:r0 // V + nch], y2v,
                                     axis=AX.X)

        # ---- combine + project:  h[c, b] = sum_d P[d, c] * (t1+g)[d, b] ----
        for h in range(2):
            nc.vector.tensor_add(t1[h][:, :], t1[h][:, :], gacc[h])

        psHS = gpsum.tile([128, 2 * BC], F32, tag="gps", name="psHS")
        psH = psHS[:, 0:BC]
        hv = psH[0:KIN, :]
        nc.tensor.matmul(hv, psc_t0[:, :], t1[0][:, :], start=True, stop=False)
        nc.tensor.matmul(hv, psc_t1[:, :], t1[1][:, :], start=False, stop=True)

        u_t = small.tile([64, BC], F32)
        nc.scalar.activation(u_t[:, :], psH[0:64, :], AF.Tanh,
                             bias=batt1_t[:, :])
        a_t = small.tile([1, BC], F32)
        # a = emb @ W_cls / V + b_cls  (adding b_cls here is fine: sum w = 1)
        nc.scalar.activation(a_t[:, :], psH[64:65, :], AF.Identity,
                             bias=bcls_t[:, :])

        psS = psHS[:, BC:2 * BC]
        nc.tensor.matmul(psS[0:1, :], watt2_t[:, :], u_t[:, :],
                         start=True, stop=True)
        s_t = small.tile([1, BC], F32)
        nc.vector.tensor_copy(s_t[:, :], psS[0:1, :])

        # ---- reshape [1, BC] -> [SC, K8] via DRAM round trip ----
        scr_s = dram_p.tile([1, BC], F32)
        scr_a = dram_p.tile([1, BC], F32)
        nc.sync.dma_start(scr_s[:, :], s_t[:, :])
        nc.sync.dma_start(scr_a[:, :], a_t[:, :])
        s32 = small.tile([SC, K8], F32)
        a32 = small.tile([SC, K8], F32)
        nc.sync.dma_start(
            s32[:, :], scr_s[:, :].rearrange("o (s k) -> (o s) k", k=K8))
        nc.sync.dma_start(
            a32[:, :], scr_a[:, :].rearrange("o (s k) -> (o s) k", k=K8))

        # ---- per-sample softmax over the 8 chunks, samples on partitions ----
        smax = small.tile([SC, 1], F32)
        nc.vector.reduce_max(smax[:, :], s32[:, :], axis=AX.X)
        es = small.tile([SC, K8], F32)
        nc.vector.tensor_scalar(es[:, :], s32[:, :], smax[:, :], None,
                                op0=ALU.subtract)
        e_t = small.tile([SC, K8], F32)
        nc.scalar.activation(e_t[:, :], es[:, :], AF.Exp)
        ssum = small.tile([SC, 1], F32)
        nc.vector.reduce_sum(ssum[:, :], e_t[:, :], axis=AX.X)
        rec = small.tile([SC, 1], F32)
        nc.vector.reciprocal(rec[:, :], ssum[:, :])
        wa = small.tile([SC, K8], F32)
        nc.vector.tensor_mul(wa[:, :], e_t[:, :], a32[:, :])
        was = small.tile([SC, 1], F32)
        nc.vector.reduce_sum(was[:, :], wa[:, :], axis=AX.X)
        o_t = small.tile([SC, 1], F32)
        nc.vector.tensor_mul(o_t[:, :], was[:, :], rec[:, :])
        nc.sync.dma_start(out[:, :], o_t[:, :])


@functools.lru_cache(maxsize=1)
def _build():
    nc = bacc.Bacc(
        "TRN2",
        target_bir_lowering=False,
        debug=False,
        enable_asserts=False,
        num_devices=NCORES,
    )
    featT = nc.dram_tensor("featT", [KIN, RC], BF16, kind="ExternalInput")
    countP = nc.dram_tensor("countP", [128, KT * BC], FP8,
                            kind="ExternalInput")
    geneP = nc.dram_tensor("geneP", [128, KT * D], BF16, kind="ExternalInput")
    w65 = nc.dram_tensor("w65", [KIN, D], BF16, kind="ExternalInput")
    bfeat = nc.dram_tensor("bfeat", [D, 1], F32, kind="ExternalInput")
    psc = nc.dram_tensor("psc", [D, KIN], F32, kind="ExternalInput")
    batt1 = nc.dram_tensor("batt1", [64, 1], F32, kind="ExternalInput")
    watt2 = nc.dram_tensor("watt2", [64, 1], F32, kind="ExternalInput")
    bcls = nc.dram_tensor("bcls", [1, 1], F32, kind="ExternalInput")
    out = nc.dram_tensor("out", [SC, 1], F32, kind="ExternalOutput")
    with tile.TileContext(nc) as tc:
        _emit(nc, tc, featT.ap(), countP.ap(), geneP.ap(), w65.ap(),
              bfeat.ap(), psc.ap(), batt1.ap(), watt2.ap(), bcls.ap(),
              out.ap())
    nc.compile()
    return nc


def _prep_inputs(features, positions, gene_ids, mask, original_sample_indices,
                 W_feat, b_feat, gene_table, w_pos,
                 W_att1, b_att1, W_att2, b_att2, W_cls, b_cls):
    features = np.asarray(features, np.float32)
    positions = np.asarray(positions)
    gene_ids = np.asarray(gene_ids)
    fp8np = mybir.dt.np(FP8)

    featT_full = np.empty((KIN, B * V), np.float32)
    featT_full[:F] = features.reshape(B * V, F).T
    featT_full[F] = positions.reshape(-1).astype(np.float32) * POS_SCALE
    featT_bf = featT_full.astype(ml_dtypes.bfloat16)

    gp = np.zeros((G_PAD, D), np.float32)
    gp[:G] = np.asarray(gene_table, np.float32)
    gene_packed = np.ascontiguousarray(
        gp.reshape(KT, 128, D).transpose(1, 0, 2).reshape(128, KT * D)
        .astype(ml_dtypes.bfloat16))

    w65v = np.concatenate(
        [np.asarray(W_feat, np.float32),
         np.asarray(w_pos, np.float32)[None, :]], axis=0
    ).astype(ml_dtypes.bfloat16)
    pscv = np.ascontiguousarray(
        np.concatenate([np.asarray(W_att1, np.float32),
                        np.asarray(W_cls, np.float32)], axis=1) / V)
    bfeatv = np.ascontiguousarray(np.asarray(b_feat, np.float32)[:, None])
    batt1v = np.ascontiguousarray(np.asarray(b_att1, np.float32)[:, None])
    watt2v = np.ascontiguousarray(np.asarray(W_att2, np.float32))
    bclsv = np.asarray(b_cls, np.float32).reshape(1, 1)

    ids = gene_ids.reshape(B, V).astype(np.int64)
    chunk_base = np.arange(BC, dtype=np.int64)[:, None] * G_PAD
    in_maps = []
    for c in range(NCORES):
        ids_c = ids[c * BC:(c + 1) * BC]
        flat = (chunk_base + ids_c).ravel()
        counts = np.bincount(flat, minlength=BC * G_PAD).reshape(BC, G_PAD)
        if counts.max() > 16:
            return None  # fp8 would be inexact; caller falls back
        counts_packed = np.ascontiguousarray(
            counts.T.reshape(KT, 128, BC).transpose(1, 0, 2)
            .reshape(128, KT * BC).astype(fp8np))
        in_maps.append({
            "featT": np.ascontiguousarray(featT_bf[:, c * RC:(c + 1) * RC]),
            "countP": counts_packed,
            "geneP": gene_packed,
            "w65": w65v,
            "bfeat": bfeatv,
            "psc": pscv,
            "batt1": batt1v,
            "watt2": watt2v,
            "bcls": bclsv,
        })
    return in_maps


def _run(inputs, trace=False, **kw):
    nc = _build()
    in_maps = _prep_inputs(**inputs)
    if in_maps is None:
        return None, None
    res = run_bass_kernel_spmd(
        nc, in_maps, core_ids=list(range(NCORES)), trace=trace, **kw)
    outv = np.concatenate(
        [np.asarray(res.results[c]["out"], np.float32) for c in range(NCORES)],
        axis=0)
    return outv, res


def _numpy_fallback(features, positions, gene_ids, mask,
                    original_sample_indices, W_feat, b_feat, gene_table,
                    w_pos, W_att1, b_att1, W_att2, b_att2, W_cls, b_cls):
    features = np.asarray(features, np.float32)
    mask_f = np.asarray(mask, np.float32)
    pos = np.asarray(positions).astype(np.float32) * POS_SCALE
    x = np.tanh(features @ np.asarray(W_feat, np.float32)
                + np.asarray(b_feat, np.float32)
                + pos[..., None] * np.asarray(w_pos, np.float32))
    x = x + np.asarray(gene_table, np.float32)[np.asarray(gene_ids)]
    denom = np.maximum(mask_f.sum(-1, keepdims=True), 1.0)
    emb = (x * mask_f[..., None]).sum(axis=1) / denom
    scores = (np.tanh(emb @ np.asarray(W_att1, np.float32)
                      + np.asarray(b_att1, np.float32))
              @ np.asarray(W_att2, np.float32)
              + np.asarray(b_att2, np.float32))[:, 0]
    seg = np.asarray(original_sample_indices).astype(np.int64)
    smax = np.full(S, -np.inf, np.float32)
    np.maximum.at(smax, seg, scores)
    e = np.exp(scores - smax[seg])
    ssum = np.zeros(S, np.float32)
    np.add.at(ssum, seg, e)
    w = e / ssum[seg]
    agg = np.zeros((S, D), np.float32)
    np.add.at(agg, seg, emb * w[:, None])
    return agg @ np.asarray(W_cls, np.float32) + np.asarray(b_cls, np.float32)


def kernel(**inputs):
    mask = np.asarray(inputs["mask"])
    seg = np.asarray(inputs["original_sample_indices"]).astype(np.int64)
    expected_seg = np.arange(B) // K8
    if not mask.all() or not np.array_equal(seg, expected_seg):
        return _numpy_fallback(**inputs)
    outv, _ = _run(inputs)
    if outv is None:
        return _numpy_fallback(**inputs)
    return outv
